# revision 1
# baseline (speedup 1.0000x reference)
"""Trainium2 Bass kernel for AttentionRNN (Bahdanau-style additive attention).

Reference computation (B=32, T=2048, D=U=1024):
    q_proj = (query @ w1 + b1)[:, None, :]          # [B, 1, U]
    k_proj = values @ w2 + b2                        # [B, T, U]
    score  = tanh(q_proj + k_proj) @ v + bv          # [B, T, 1]
    attention_weights = softmax(score, axis=1)       # [B, T, 1]
    context = sum(attention_weights * values, axis=1)  # [B, D]
    returns (context, attention_weights)

Sharding: batch B split across 8 NeuronCores (4 examples/core); w2/v/q_proj
replicated/precomputed on host (tiny). Per core the dominant work is
k_proj = values @ w2 (17.2 GFLOP) done in bf16 on the PE at ~78 TF/s, with
values transposed on-chip via the XBAR DMA-transpose (d onto partitions).
bv is a constant shift of the scores -> softmax-invariant -> dropped.
"""

import numpy as np
import ml_dtypes

B, T, D, U = 32, 2048, 1024, 1024
N_CORES = 8
BL = B // N_CORES          # 4 examples per core
P = 128
NT = T // P                # 16 t-slabs of 128 per example
TC = 512                   # t-chunk for the main matmul
NCH = T // TC              # 4 chunks per example
SLABS_PER_CHUNK = TC // P  # 4
DT = D // P                # 8 d-tiles
UT = U // P                # 8 u-tiles
LOAD_GROUP = 4             # t-slabs per HBM load DMA (2 MB each)

_cache = {}


def _build():
    import concourse.bass as bass
    import concourse.mybir as mybir
    import concourse.tile as tile
    from concourse import bacc

    f32 = mybir.dt.float32
    bf16 = mybir.dt.bfloat16
    AF = mybir.ActivationFunctionType

    nc = bacc.Bacc("TRN2", target_bir_lowering=False, debug=False,
                   num_devices=N_CORES)

    # Per-core inputs
    vals = nc.dram_tensor("vals", [BL, T, D], f32, kind="ExternalInput")
    w2s = nc.dram_tensor("w2s", [P, DT * UT * P], bf16, kind="ExternalInput")
    qbt = nc.dram_tensor("qbt", [P, UT * BL], f32, kind="ExternalInput")
    v8 = nc.dram_tensor("v8", [P, UT], bf16, kind="ExternalInput")
    # Per-core outputs
    ctx_o = nc.dram_tensor("ctx", [BL, D], f32, kind="ExternalOutput")
    aw_o = nc.dram_tensor("aw", [BL, T], f32, kind="ExternalOutput")
    # DRAM scratch for the weight partition-relayout roundtrip
    wscr = nc.dram_tensor("wscr", [BL, T], bf16)

    vals_r = vals.ap().rearrange("b (n p) d -> b n p d", p=P)  # [BL, NT, 128, D]

    with tile.TileContext(nc) as tc:
        with (
            tc.tile_pool(name="consts", bufs=1) as consts,
            tc.tile_pool(name="stage", bufs=2) as stage_p,
            tc.tile_pool(name="v16", bufs=2) as v16_p,
            tc.tile_pool(name="vt", bufs=2) as vt_p,
            tc.tile_pool(name="tanh", bufs=10) as tanh_p,
            tc.tile_pool(name="sc", bufs=2) as sc_p,
            tc.tile_pool(name="small", bufs=3) as small_p,
            tc.tile_pool(name="w16", bufs=2) as w16_p,
            tc.tile_pool(name="wst", bufs=2) as wst_p,
            tc.tile_pool(name="cv", bufs=2) as cv_p,
            tc.tile_pool(name="psK", bufs=3, space="PSUM") as psK_p,
            tc.tile_pool(name="psS", bufs=2, space="PSUM") as psS_p,
            tc.tile_pool(name="psC", bufs=1, space="PSUM") as psC_p,
        ):
            w2_sb = consts.tile([P, DT * UT * P], bf16)
            nc.sync.dma_start(out=w2_sb, in_=w2s.ap())
            w2_v = w2_sb.rearrange("p (a b c) -> p a b c", b=UT, c=P)
            qb_sb = consts.tile([P, UT * BL], f32)
            nc.sync.dma_start(out=qb_sb, in_=qbt.ap())
            qb_v = qb_sb.rearrange("p (a b) -> p a b", b=BL)
            v8_sb = consts.tile([P, UT], bf16)
            nc.sync.dma_start(out=v8_sb, in_=v8.ap())
            # score staging: partition b holds example b's (unnormalized) bf16
            # softmax weights; padded to 16 partitions for the XBAR transpose
            scp = consts.tile([16, T], bf16)
            nc.vector.memset(scp, 0.0)

            def phase_a(b):
                """scores + softmax for example b; returns (v16, rinv)."""
                v16 = v16_p.tile([P, NT, D], bf16)
                for g in range(NT // LOAD_GROUP):
                    st = stage_p.tile([P, LOAD_GROUP, D], f32)
                    src = vals_r[b, g * LOAD_GROUP:(g + 1) * LOAD_GROUP]
                    nc.sync.dma_start(out=st, in_=src.rearrange("n p d -> p n d"))
                    nc.vector.tensor_copy(
                        out=v16[:, g * LOAD_GROUP:(g + 1) * LOAD_GROUP, :], in_=st)

                sc = sc_p.tile([1, T], f32)
                for c in range(NCH):
                    vt = vt_p.tile([P, DT, TC], bf16)
                    for h in range(SLABS_PER_CHUNK):
                        s = c * SLABS_PER_CHUNK + h
                        nc.sync.dma_start(out=vt[:, :, h * P:(h + 1) * P],
                                          in_=v16[:, s, :], transpose=True)
                    pS = psS_p.tile([1, TC], f32)
                    ths = []
                    for ut in range(UT):
                        pK = psK_p.tile([P, TC], f32)
                        for dt in range(DT):
                            nc.tensor.matmul(pK, w2_v[:, dt, ut, :], vt[:, dt, :],
                                             start=(dt == 0), stop=(dt == DT - 1))
                        th = tanh_p.tile([P, TC], bf16)
                        nc.scalar.activation(th, pK, AF.Tanh,
                                             bias=qb_v[:, ut, b:b + 1])
                        ths.append(th)
                    for ut in range(UT):
                        nc.tensor.matmul(pS, v8_sb[:, ut:ut + 1], ths[ut],
                                         start=(ut == 0), stop=(ut == UT - 1))
                    nc.scalar.copy(sc[:, c * TC:(c + 1) * TC], pS)

                # softmax over T (all on partition 0)
                m = small_p.tile([1, 1], f32)
                nc.vector.tensor_reduce(m, sc, axis=mybir.AxisListType.X,
                                        op=mybir.AluOpType.max, negate=True)
                den = small_p.tile([1, 1], f32)
                nc.scalar.activation(sc, sc, AF.Exp, bias=m, accum_out=den)
                rinv = small_p.tile([1, 1], f32)
                nc.vector.reciprocal(rinv, den)
                w16 = w16_p.tile([1, T], bf16)
                nc.vector.tensor_copy(w16, sc)          # unnormalized exp, bf16
                awn = sc_p.tile([1, T], f32, tag="awn")
                nc.vector.tensor_scalar_mul(awn, sc, rinv)
                nc.sync.dma_start(out=aw_o.ap()[b:b + 1, :], in_=awn)
                # roundtrip through DRAM to land the weights on partition b
                # (avoids plain SBUF->SBUF DMA next to XBAR transposes)
                nc.sync.dma_start(out=wscr.ap()[b:b + 1, :], in_=w16)
                nc.sync.dma_start(out=scp[b:b + 1, :], in_=wscr.ap()[b:b + 1, :])
                return v16, rinv

            def phase_b(b, v16, rinv):
                """context vector for example b."""
                wst = wst_p.tile([P, NT, 16], bf16)
                nc.sync.dma_start(out=wst, in_=scp, transpose=True)
                pc0 = psC_p.tile([1, 512], f32, tag="pc0")
                pc1 = psC_p.tile([1, 512], f32, tag="pc1")
                for s in range(NT):
                    nc.tensor.matmul(pc0, wst[:, s, b:b + 1], v16[:, s, 0:512],
                                     start=(s == 0), stop=(s == NT - 1))
                    nc.tensor.matmul(pc1, wst[:, s, b:b + 1], v16[:, s, 512:1024],
                                     start=(s == 0), stop=(s == NT - 1))
                cv = cv_p.tile([1, D], f32)
                nc.scalar.mul(cv[:, 0:512], pc0, rinv)
                nc.scalar.mul(cv[:, 512:1024], pc1, rinv)
                nc.sync.dma_start(out=ctx_o.ap()[b:b + 1, :], in_=cv)

            # software pipeline: A0 A1 B0 A2 B1 A3 B2 B3
            pend = []
            for b in range(BL):
                pend.append((b, *phase_a(b)))
                if b >= 1:
                    phase_b(*pend.pop(0))
            while pend:
                phase_b(*pend.pop(0))

    nc.compile()
    return nc


def _get_nc():
    if "nc" not in _cache:
        _cache["nc"] = _build()
    return _cache["nc"]


def kernel(query, values, w1, b1, w2, b2, v, bv):
    from concourse.bass_utils import run_bass_kernel_spmd

    query = np.asarray(query, np.float32)
    values = np.ascontiguousarray(np.asarray(values, np.float32))
    w1 = np.asarray(w1, np.float32)
    b1 = np.asarray(b1, np.float32)
    w2 = np.asarray(w2, np.float32)
    b2 = np.asarray(b2, np.float32)
    v = np.asarray(v, np.float32)
    # bv only shifts scores uniformly -> softmax output unchanged; dropped.

    bf = ml_dtypes.bfloat16
    # host prep (tiny): q-projection folded with both biases, per-(b,u) bias
    qb = query @ w1 + b1 + b2                                   # [B, U]
    qbt_full = np.ascontiguousarray(
        qb.T.reshape(UT, P, B).transpose(1, 0, 2))              # [128, UT, B]
    w2s = np.ascontiguousarray(
        w2.reshape(DT, P, UT, P).transpose(1, 0, 2, 3)
        .reshape(P, DT * UT * P).astype(bf))                    # [128, DT*UT*128]
    v8 = np.ascontiguousarray(v[:, 0].reshape(UT, P).T.astype(bf))  # [128, UT]

    nc = _get_nc()
    in_maps = []
    for i in range(N_CORES):
        bs = slice(i * BL, (i + 1) * BL)
        in_maps.append({
            "vals": np.ascontiguousarray(values[bs]),
            "w2s": w2s,
            "qbt": np.ascontiguousarray(
                qbt_full[:, :, bs].reshape(P, UT * BL)),
            "v8": v8,
        })
    res = run_bass_kernel_spmd(nc, in_maps, core_ids=list(range(N_CORES)),
                               **_cache.get("run_kwargs", {}))
    _cache["last_results"] = res

    context = np.concatenate([res.results[i]["ctx"] for i in range(N_CORES)], 0)
    aw = np.concatenate([res.results[i]["aw"] for i in range(N_CORES)], 0)
    return context.astype(np.float32), aw.reshape(B, T, 1).astype(np.float32)


# revision 2
# speedup vs baseline: 1.2314x; 1.2314x over previous
"""Trainium2 Bass kernel for AttentionRNN (Bahdanau-style additive attention).

Reference computation (B=32, T=2048, D=U=1024):
    q_proj = (query @ w1 + b1)[:, None, :]          # [B, 1, U]
    k_proj = values @ w2 + b2                        # [B, T, U]
    score  = tanh(q_proj + k_proj) @ v + bv          # [B, T, 1]
    attention_weights = softmax(score, axis=1)       # [B, T, 1]
    context = sum(attention_weights * values, axis=1)  # [B, D]
    returns (context, attention_weights)

Sharding: batch B split across 8 NeuronCores (4 examples/core); w2/v and the
q-projection (computed on host, it is tiny) are replicated. Per core the
dominant work is k_proj = values @ w2 (17.2 GFLOP) done in bf16 on the PE,
with values transposed on-chip via the XBAR DMA-transpose so the contraction
dim (d) lands on partitions. bv shifts all scores uniformly -> softmax
invariant -> dropped.

Queue split to avoid head-of-line blocking: SWDGE (gpsimd) carries the big
HBM loads and the small softmax/output DMAs; the sync HWDGE queue carries
only XBAR transposes (+ constants at startup).
"""

import numpy as np
import ml_dtypes

B, T, D, U = 32, 2048, 1024, 1024
N_CORES = 8
BL = B // N_CORES          # 4 examples per core
P = 128
NT = T // P                # 16 t-slabs of 128 per example
TC = 512                   # t-chunk for the main matmul
NCH = T // TC              # 4 chunks per example
SLABS_PER_CHUNK = TC // P  # 4
DT = D // P                # 8 d-tiles
UT = U // P                # 8 u-tiles
LOAD_GROUP = 4             # t-slabs per HBM load DMA (1 MB each)

_cache = {}


def _build():
    import concourse.bass as bass
    import concourse.mybir as mybir
    import concourse.tile as tile
    from concourse import bacc

    f32 = mybir.dt.float32
    bf16 = mybir.dt.bfloat16
    AF = mybir.ActivationFunctionType

    nc = bacc.Bacc("TRN2", target_bir_lowering=False, debug=False,
                   num_devices=N_CORES)

    # Per-core inputs (values pre-cast to bf16 on host)
    vals = nc.dram_tensor("vals", [BL, T, D], bf16, kind="ExternalInput")
    w2s = nc.dram_tensor("w2s", [P, DT * UT * P], bf16, kind="ExternalInput")
    qbt = nc.dram_tensor("qbt", [P, UT * BL], f32, kind="ExternalInput")
    v8 = nc.dram_tensor("v8", [P, UT], bf16, kind="ExternalInput")
    # Per-core outputs
    ctx_o = nc.dram_tensor("ctx", [BL, D], f32, kind="ExternalOutput")
    aw_o = nc.dram_tensor("aw", [BL, T], f32, kind="ExternalOutput")
    # DRAM scratch for the weight partition-relayout roundtrip
    wscr = nc.dram_tensor("wscr", [BL, T], bf16)

    vals_r = vals.ap().rearrange("b (n p) d -> b n p d", p=P)  # [BL, NT, 128, D]

    with tile.TileContext(nc) as tc:
        with (
            tc.tile_pool(name="consts", bufs=1) as consts,
            tc.tile_pool(name="v16", bufs=3) as v16_p,
            tc.tile_pool(name="vt", bufs=2) as vt_p,
            tc.tile_pool(name="tanh", bufs=10) as tanh_p,
            tc.tile_pool(name="sc", bufs=2) as sc_p,
            tc.tile_pool(name="small", bufs=3) as small_p,
            tc.tile_pool(name="w16", bufs=2) as w16_p,
            tc.tile_pool(name="wst", bufs=2) as wst_p,
            tc.tile_pool(name="cv", bufs=2) as cv_p,
            tc.tile_pool(name="psK", bufs=3, space="PSUM") as psK_p,
            tc.tile_pool(name="psS", bufs=2, space="PSUM") as psS_p,
            tc.tile_pool(name="psC", bufs=1, space="PSUM") as psC_p,
        ):
            w2_sb = consts.tile([P, DT * UT * P], bf16)
            nc.sync.dma_start(out=w2_sb, in_=w2s.ap())
            w2_v = w2_sb.rearrange("p (a b c) -> p a b c", b=UT, c=P)
            qb_sb = consts.tile([P, UT * BL], f32)
            nc.sync.dma_start(out=qb_sb, in_=qbt.ap())
            qb_v = qb_sb.rearrange("p (a b) -> p a b", b=BL)
            v8_sb = consts.tile([P, UT], bf16)
            nc.sync.dma_start(out=v8_sb, in_=v8.ap())
            # score staging: partition b holds example b's (unnormalized) bf16
            # softmax weights; padded to 16 partitions for the XBAR transpose
            scp = consts.tile([16, T], bf16)
            nc.vector.memset(scp, 0.0)

            def load_example(b):
                """HBM -> SBUF bf16 values for example b (SWDGE queue)."""
                v16 = v16_p.tile([P, NT, D], bf16)
                for g in range(NT // LOAD_GROUP):
                    src = vals_r[b, g * LOAD_GROUP:(g + 1) * LOAD_GROUP]
                    nc.gpsimd.dma_start(
                        out=v16[:, g * LOAD_GROUP:(g + 1) * LOAD_GROUP, :],
                        in_=src.rearrange("n p d -> p n d"))
                return v16

            def compute_a(b, v16):
                """scores + softmax for example b; returns rinv."""
                sc = sc_p.tile([1, T], f32)
                for c in range(NCH):
                    vt = vt_p.tile([P, DT, TC], bf16)
                    for h in range(SLABS_PER_CHUNK):
                        s = c * SLABS_PER_CHUNK + h
                        nc.sync.dma_start(out=vt[:, :, h * P:(h + 1) * P],
                                          in_=v16[:, s, :], transpose=True)
                    pS = psS_p.tile([1, TC], f32)
                    ths = []
                    for ut in range(UT):
                        pK = psK_p.tile([P, TC], f32)
                        for dt in range(DT):
                            nc.tensor.matmul(pK, w2_v[:, dt, ut, :], vt[:, dt, :],
                                             start=(dt == 0), stop=(dt == DT - 1))
                        th = tanh_p.tile([P, TC], bf16)
                        nc.scalar.activation(th, pK, AF.Tanh,
                                             bias=qb_v[:, ut, b:b + 1])
                        ths.append(th)
                    for ut in range(UT):
                        nc.tensor.matmul(pS, v8_sb[:, ut:ut + 1], ths[ut],
                                         start=(ut == 0), stop=(ut == UT - 1))
                    nc.scalar.copy(sc[:, c * TC:(c + 1) * TC], pS)

                # softmax over T (all on partition 0)
                m = small_p.tile([1, 1], f32)
                nc.vector.tensor_reduce(m, sc, axis=mybir.AxisListType.X,
                                        op=mybir.AluOpType.max, negate=True)
                den = small_p.tile([1, 1], f32)
                nc.scalar.activation(sc, sc, AF.Exp, bias=m, accum_out=den)
                rinv = small_p.tile([1, 1], f32)
                nc.vector.reciprocal(rinv, den)
                w16 = w16_p.tile([1, T], bf16)
                nc.vector.tensor_copy(w16, sc)          # unnormalized exp, bf16
                nc.vector.tensor_scalar_mul(sc, sc, rinv)   # normalize in place
                nc.gpsimd.dma_start(out=aw_o.ap()[b:b + 1, :], in_=sc)
                # roundtrip through DRAM to land the weights on partition b
                # (avoids plain SBUF->SBUF DMA next to XBAR transposes)
                nc.gpsimd.dma_start(out=wscr.ap()[b:b + 1, :], in_=w16)
                nc.gpsimd.dma_start(out=scp[b:b + 1, :], in_=wscr.ap()[b:b + 1, :])
                return rinv

            def phase_b(b, v16, rinv):
                """context vector for example b."""
                wst = wst_p.tile([P, NT, 16], bf16)
                nc.sync.dma_start(out=wst, in_=scp, transpose=True)
                pc0 = psC_p.tile([1, 512], f32, tag="pc0")
                pc1 = psC_p.tile([1, 512], f32, tag="pc1")
                for s in range(NT):
                    nc.tensor.matmul(pc0, wst[:, s, b:b + 1], v16[:, s, 0:512],
                                     start=(s == 0), stop=(s == NT - 1))
                    nc.tensor.matmul(pc1, wst[:, s, b:b + 1], v16[:, s, 512:1024],
                                     start=(s == 0), stop=(s == NT - 1))
                cv = cv_p.tile([1, D], f32)
                nc.scalar.mul(cv[:, 0:512], pc0, rinv)
                nc.scalar.mul(cv[:, 512:1024], pc1, rinv)
                nc.gpsimd.dma_start(out=ctx_o.ap()[b:b + 1, :], in_=cv)

            # software pipeline (loads prefetched 2 examples ahead):
            # L0 L1 | A0{L2} A1{L3} B0 A2 B1 A3 B2 B3
            v16s = [None] * BL
            v16s[0] = load_example(0)
            if BL > 1:
                v16s[1] = load_example(1)
            pend = []
            for b in range(BL):
                if b + 2 < BL:
                    v16s[b + 2] = load_example(b + 2)
                rinv = compute_a(b, v16s[b])
                pend.append((b, v16s[b], rinv))
                if b >= 1:
                    phase_b(*pend.pop(0))
            while pend:
                phase_b(*pend.pop(0))

    nc.compile()
    return nc


def _get_nc():
    if "nc" not in _cache:
        _cache["nc"] = _build()
    return _cache["nc"]


def kernel(query, values, w1, b1, w2, b2, v, bv):
    from concourse.bass_utils import run_bass_kernel_spmd

    query = np.asarray(query, np.float32)
    values = np.asarray(values, np.float32)
    w1 = np.asarray(w1, np.float32)
    b1 = np.asarray(b1, np.float32)
    w2 = np.asarray(w2, np.float32)
    b2 = np.asarray(b2, np.float32)
    v = np.asarray(v, np.float32)
    # bv only shifts scores uniformly -> softmax output unchanged; dropped.

    bf = ml_dtypes.bfloat16
    # host prep (tiny except the values cast): q-projection folded with both
    # biases, weight relayouts, values -> bf16
    qb = query @ w1 + b1 + b2                                   # [B, U]
    qbt_full = np.ascontiguousarray(
        qb.T.reshape(UT, P, B).transpose(1, 0, 2))              # [128, UT, B]
    w2s = np.ascontiguousarray(
        w2.reshape(DT, P, UT, P).transpose(1, 0, 2, 3)
        .reshape(P, DT * UT * P).astype(bf))                    # [128, DT*UT*128]
    v8 = np.ascontiguousarray(v[:, 0].reshape(UT, P).T.astype(bf))  # [128, UT]
    vals16 = np.ascontiguousarray(values.astype(bf))            # [B, T, D]

    nc = _get_nc()
    in_maps = []
    for i in range(N_CORES):
        bs = slice(i * BL, (i + 1) * BL)
        in_maps.append({
            "vals": vals16[bs],
            "w2s": w2s,
            "qbt": np.ascontiguousarray(
                qbt_full[:, :, bs].reshape(P, UT * BL)),
            "v8": v8,
        })
    res = run_bass_kernel_spmd(nc, in_maps, core_ids=list(range(N_CORES)),
                               **_cache.get("run_kwargs", {}))
    _cache["last_results"] = res

    context = np.concatenate([res.results[i]["ctx"] for i in range(N_CORES)], 0)
    aw = np.concatenate([res.results[i]["aw"] for i in range(N_CORES)], 0)
    return context.astype(np.float32), aw.reshape(B, T, 1).astype(np.float32)
